# revision 17
# baseline (speedup 1.0000x reference)
"""Trainium2 Bass kernel for BeeSenseSelector (topk channel masking).

reference semantics:
    pooled = mean(x, axis=(1,2))               # [B, C]
    scores = sigmoid(pooled @ W + b)           # [B, C]
    mask   = top_k(scores, C//2) scatter 1.0   # [B, C]
    out    = x * mask[:, None, None, :]

Strategy (8 cores x 4 samples, data-parallel over batch; single pass over x):
  - x[s] viewed as [12544, 256] -> SBUF chunks [128 part, 14, 256] (partition
    p owns spatial rows p*98..p*98+97); 7 chunks per sample. 1.75MB read /
    0.9MB write DMAs stay near peak HBM efficiency.
  - pooling is exact fp32 (the min top-k score gap on this data is 3.4e-6 in
    z, so reduced-precision pooling risks flipping the mask): DVE folds the
    14 chunk rows to 7 (one tensor_add), PE ones-matmuls accumulate those 7
    rows over partitions into pooled [1, C] PSUM (fp32 rhs, exact).
  - scalar engine converts each x chunk to bf16 right after the fold; the
    f32 chunk is then freed (decouples load prefetch from the mask latency).
  - gating on PE: transpose pooled row -> [ci, 1], matmul with W chunks,
    sigmoid w/ scale=1/HW and bias=b -> scoresT [128, 2] in SBUF.
  - rank-based exact top-k (ties broken by lower index, like lax.top_k):
      rank[f] = #{p: s[p] > s[f]} + #{p < f: s[p] == s[f]},  mask = rank < K
    via DVE compares against a PE-broadcast of scores + ones-matmul count.
  - multiply: DVE bf16 x bf16 in-place (2x DVE mode) with the bf16 mask
    broadcast; store bf16.  Output dtype is bf16 (harness gate is
    rel_err < 2e-2; bf16 rounding is ~2e-3): write traffic halves, so
    HBM/core is 51.4 MB read + 25.7 MB write.
  - software pipelining at chunk granularity: sample s-1's mul+store for
    chunk j are issued right after sample s's chunk-j load/fold/convert, so
    the in-order DVE frees f32 bufs steadily and reads never stall. Store
    DMAs dispatch from the scalar engine (the second HWDGE ring), so the
    sync queue carries only loads.
"""

import numpy as np

B, H, W_, C = 32, 112, 112, 256
KTOP = C // 2
NCORES = 8
NPC = B // NCORES          # samples per core
S = H * W_                 # 12544 spatial positions
P = 128                    # partitions
ROWS = S // P              # 98 spatial rows per partition
CH = 14                    # rows per chunk
NCH = ROWS // CH           # 7 chunks
XBUFS = 5                  # f32 x-tile slots (14KB/partition each)
BBUFS = 14                 # bf16 x-tile slots (7KB/partition each)


def build(nc, n_samples=NPC):
    import concourse.tile as tile
    import concourse.mybir as mybir
    from contextlib import ExitStack

    f32 = mybir.dt.float32
    bf16 = mybir.dt.bfloat16
    Alu = mybir.AluOpType

    x_d = nc.dram_tensor("x", [n_samples, H, W_, C], f32, kind="ExternalInput")
    w_d = nc.dram_tensor("W", [C, C], f32, kind="ExternalInput")
    b_d = nc.dram_tensor("b", [C], f32, kind="ExternalInput")
    o_d = nc.dram_tensor("out", [n_samples, H, W_, C], bf16,
                         kind="ExternalOutput")

    # constants baked into the NEFF
    pidx = np.arange(P)[:, None, None] + 128 * np.arange(2)[None, :, None]
    ut_np = (pidx < np.arange(C)[None, None, :]).astype(np.float32)  # [128, 2, 256]
    ut_d = nc.inline_tensor(ut_np, name="ut_const")
    id_d = nc.inline_tensor(np.eye(P, dtype=np.float32), name="id_const")

    x_v = x_d.ap().rearrange("s h w c -> s (h w) c").rearrange(
        "s (p n) c -> s p n c", p=P)
    o_v = o_d.ap().rearrange("s h w c -> s (h w) c").rearrange(
        "s (p n) c -> s p n c", p=P)

    with tile.TileContext(nc) as tc, ExitStack() as ctx:
        cst = ctx.enter_context(tc.tile_pool(name="cst", bufs=1))
        xp = ctx.enter_context(tc.tile_pool(name="xp", bufs=XBUFS))
        xb = ctx.enter_context(tc.tile_pool(name="xb", bufs=BBUFS))
        sm = ctx.enter_context(tc.tile_pool(name="sm", bufs=2))

        ps_pr = ctx.enter_context(tc.tile_pool(name="ps_pr", bufs=1, space="PSUM"))
        ps_t2 = ctx.enter_context(tc.tile_pool(name="ps_t2", bufs=1, space="PSUM"))
        ps_zt0 = ctx.enter_context(tc.tile_pool(name="ps_zt0", bufs=1, space="PSUM"))
        ps_zt1 = ctx.enter_context(tc.tile_pool(name="ps_zt1", bufs=1, space="PSUM"))
        ps_tr = ctx.enter_context(tc.tile_pool(name="ps_tr", bufs=1, space="PSUM"))
        ps_sb = ctx.enter_context(tc.tile_pool(name="ps_sb", bufs=1, space="PSUM"))
        ps_rk = ctx.enter_context(tc.tile_pool(name="ps_rk", bufs=1, space="PSUM"))
        ps_mb = ctx.enter_context(tc.tile_pool(name="ps_mb", bufs=1, space="PSUM"))

        # constants go out on the scalar-engine DMA queue so the first x
        # chunk reads (sync queue) dispatch immediately
        w_sb = cst.tile([P, 2, C], f32)
        nc.scalar.dma_start(w_sb, w_d.ap().rearrange("(h p) c -> p h c", p=P))
        b_sb = cst.tile([P, 2], f32)
        nc.scalar.dma_start(b_sb, b_d.ap().rearrange("(h p) -> p h", p=P))
        ut_sb = cst.tile([P, 2, C], f32)
        nc.scalar.dma_start(ut_sb, ut_d.ap())
        id_sb = cst.tile([P, P], f32)
        nc.scalar.dma_start(id_sb, id_d.ap())
        ones_c = cst.tile([P, 1], f32)
        nc.vector.memset(ones_c, 1.0)
        ones_r = cst.tile([1, P], f32)
        nc.vector.memset(ones_r, 1.0)
        ones_rb = cst.tile([1, P], bf16)
        nc.vector.memset(ones_rb, 1.0)

        pend = None  # deferred (xbs, mb16, s) of the previous sample

        def flush_one(pend, j):
            xbs, mb16, ps = pend
            mb_bc = mb16.unsqueeze(1).broadcast_to([P, CH, C])
            nc.vector.tensor_mul(xbs[j], xbs[j], mb_bc)
            nc.scalar.dma_start(o_v[ps, :, j * CH:(j + 1) * CH, :], xbs[j])

        for s in range(n_samples):
            # ---- A(s): per chunk: load; fold 14 rows -> 7 on DVE; 7 PE
            #      accumulate matmuls; bf16 convert on scalar engine; then
            #      the previous sample's chunk-j mul + store ----
            xbs = []
            pr = ps_pr.tile([1, C], f32, name=f"pr_{s}", tag="pr")
            for j in range(NCH):
                xc = xp.tile([P, CH, C], f32, tag="x", name=f"x_{s}_{j}")
                nc.sync.dma_start(xc, x_v[s, :, j * CH:(j + 1) * CH, :])
                a7 = sm.tile([P, 7, C], f32, name=f"a7_{s}_{j}", tag="a7")
                nc.vector.tensor_add(a7, xc[:, 0:7, :], xc[:, 7:14, :])
                for r in range(7):
                    nc.tensor.matmul(pr, lhsT=ones_c, rhs=a7[:, r, :],
                                     start=(j == 0 and r == 0),
                                     stop=(j == NCH - 1 and r == 6))
                xc16 = xb.tile([P, CH, C], bf16, tag="xb", name=f"xb_{s}_{j}")
                nc.scalar.copy(xc16, xc)
                xbs.append(xc16)
                if pend is not None:
                    flush_one(pend, j)

            # ---- B(s): mask chain ----
            # high_priority: the chain's small ops must not queue behind the
            # NEXT sample's converts on the scalar engine (a convert that
            # waits on its chunk read would head-of-line-block the ready
            # sigmoid/copies, delaying mask(s) and serializing the endgame)
            hp = tc.high_priority(offset=200)
            hp.__enter__()
            prow = sm.tile([1, C], f32, name=f"prow_{s}", tag="prow")
            nc.scalar.copy(prow, pr)
            t2 = ps_t2.tile([P, 2], f32, name=f"t2_{s}", tag="t2")
            for h in range(2):
                nc.tensor.transpose(t2[:, h:h + 1], prow[:, h * P:(h + 1) * P],
                                    id_sb[0:1, 0:1])
            pts = sm.tile([P, 2], f32, name=f"pts_{s}", tag="pts")
            nc.scalar.copy(pts, t2)

            # gating: zT[co_h] = sum_ci W[ci, co].T @ pooledT
            zt = [ps_zt0.tile([P, 1], f32, name=f"zt0_{s}", tag="zt0"),
                  ps_zt1.tile([P, 1], f32, name=f"zt1_{s}", tag="zt1")]
            for co in range(2):
                for ci in range(2):
                    nc.tensor.matmul(
                        zt[co],
                        lhsT=w_sb[:, ci, co * P:(co + 1) * P],
                        rhs=pts[:, ci:ci + 1],
                        start=(ci == 0),
                        stop=(ci == 1),
                    )
            st = sm.tile([P, 2], f32, name=f"st_{s}", tag="st")
            for h in range(2):
                nc.scalar.activation(
                    st[:, h:h + 1], zt[h],
                    func=mybir.ActivationFunctionType.Sigmoid,
                    bias=b_sb[:, h:h + 1], scale=1.0 / S)

            # scores row form + broadcast across partitions:
            # SB[p, h*128+i] = s[h*128+i]  (per-half transpose so the
            # matmul rhs sits at base partition 0)
            sb_ps = ps_sb.tile([P, C], f32, name=f"sb_{s}", tag="sbb")
            for h in range(2):
                tr_ps = ps_tr.tile([1, P], f32, name=f"trp_{s}_{h}", tag="trp")
                nc.tensor.transpose(tr_ps, st[:, h:h + 1], id_sb)
                tr_sb = sm.tile([1, P], f32, name=f"trs_{s}_{h}", tag="trs")
                nc.scalar.copy(tr_sb, tr_ps)
                nc.tensor.matmul(sb_ps[:, h * P:(h + 1) * P], lhsT=ones_r,
                                 rhs=tr_sb, start=True, stop=True)

            # rank comparisons
            r_sb = sm.tile([P, 2, C], f32, name=f"r_{s}", tag="r")
            eq_sb = sm.tile([P, C], f32, name=f"eq_{s}", tag="eq")
            for h in range(2):
                nc.vector.tensor_scalar(
                    r_sb[:, h, :], sb_ps, st[:, h:h + 1], None, Alu.is_lt)
                nc.vector.tensor_scalar(
                    eq_sb, sb_ps, st[:, h:h + 1], None, Alu.is_equal)
                nc.vector.tensor_mul(eq_sb, eq_sb, ut_sb[:, h, :])
                nc.vector.tensor_add(r_sb[:, h, :], r_sb[:, h, :], eq_sb)

            rk_ps = ps_rk.tile([1, C], f32, name=f"rk_{s}", tag="rk")
            for h in range(2):
                nc.tensor.matmul(rk_ps, lhsT=ones_c, rhs=r_sb[:, h, :],
                                 start=(h == 0), stop=(h == 1))

            # mask row in bf16 ({0,1} exact) so the broadcast matmul streams
            # at 1 cycle/row instead of fp32's 4
            mrow = sm.tile([1, C], bf16, name=f"mrow_{s}", tag="mrow")
            nc.vector.tensor_scalar(mrow, rk_ps, float(KTOP) - 0.5, None, Alu.is_lt)

            mb_ps = ps_mb.tile([P, C], f32, name=f"mb_{s}", tag="mb")
            nc.tensor.matmul(mb_ps, lhsT=ones_rb, rhs=mrow,
                             start=True, stop=True)
            mb16 = sm.tile([P, C], bf16, name=f"mbs_{s}", tag="mbs")
            nc.scalar.copy(mb16, mb_ps)
            hp.__exit__(None, None, None)

            pend = (xbs, mb16, s)

        for j in range(NCH):
            flush_one(pend, j)

    return nc


def make_nc(n_samples=NPC, num_devices=NCORES):
    import concourse.bacc as bacc
    nc = bacc.Bacc("TRN2", target_bir_lowering=False, debug=False,
                   num_devices=num_devices)
    build(nc, n_samples)
    nc.compile()
    return nc


_NC_CACHE = {}


def kernel(x, W, b):
    from concourse import bass_utils
    x = np.ascontiguousarray(x, dtype=np.float32)
    W = np.ascontiguousarray(W, dtype=np.float32)
    b = np.ascontiguousarray(b, dtype=np.float32)
    assert x.shape == (B, H, W_, C)
    if "nc" not in _NC_CACHE:
        _NC_CACHE["nc"] = make_nc()
    nc = _NC_CACHE["nc"]
    in_maps = [
        {"x": x[c * NPC:(c + 1) * NPC], "W": W, "b": b} for c in range(NCORES)
    ]
    # the axon terminal occasionally reports a transient
    # NRT_EXEC_UNIT_UNRECOVERABLE; a retry has always recovered it
    last_err = None
    for _ in range(3):
        try:
            res = bass_utils.run_bass_kernel_spmd(
                nc, in_maps, core_ids=list(range(NCORES)))
            return np.concatenate(
                [r["out"].astype(np.float32) for r in res.results], axis=0)
        except Exception as e:
            last_err = e
    raise last_err


# revision 19
# speedup vs baseline: 1.2068x; 1.2068x over previous
"""Trainium2 Bass kernel for BeeSenseSelector (topk channel masking).

reference semantics:
    pooled = mean(x, axis=(1,2))               # [B, C]
    scores = sigmoid(pooled @ W + b)           # [B, C]
    mask   = top_k(scores, C//2) scatter 1.0   # [B, C]
    out    = x * mask[:, None, None, :]

Strategy (8 cores x 4 samples, data-parallel over batch; single pass over x):
  - x[s] viewed as [12544, 256] -> SBUF chunks [128 part, 14, 256] (partition
    p owns spatial rows p*98..p*98+97); 7 chunks per sample. 1.75MB read /
    0.9MB write DMAs stay near peak HBM efficiency.
  - pooling is exact fp32 (the min top-k score gap on this data is 3.4e-6 in
    z, so reduced-precision pooling risks flipping the mask): DVE folds the
    14 chunk rows to 7 (one tensor_add), PE ones-matmuls accumulate those 7
    rows over partitions into pooled [1, C] PSUM (fp32 rhs, exact).
  - scalar engine converts each x chunk to bf16 right after the fold; the
    f32 chunk is then freed (decouples load prefetch from the mask latency).
  - gating on PE: transpose pooled row -> [ci, 1], matmul with W chunks,
    sigmoid w/ scale=1/HW and bias=b -> scoresT [128, 2] in SBUF.
  - rank-based exact top-k (ties broken by lower index, like lax.top_k):
      rank[f] = #{p: s[p] > s[f]} + #{p < f: s[p] == s[f]},  mask = rank < K
    via DVE compares against a PE-broadcast of scores + ones-matmul count.
  - multiply: DVE bf16 x bf16 in-place (2x DVE mode) with the bf16 mask
    broadcast; store bf16.  Output dtype is bf16 (harness gate is
    rel_err < 2e-2; bf16 rounding is ~2e-3): write traffic halves, so
    HBM/core is 51.4 MB read + 25.7 MB write.
  - software pipelining at chunk granularity: sample s-1's mul+store for
    chunk j are issued right after sample s's chunk-j load/fold/convert, so
    the in-order DVE frees f32 bufs steadily and reads never stall. Store
    DMAs dispatch from the scalar engine (the second HWDGE ring), so the
    sync queue carries only loads.
"""

import numpy as np

B, H, W_, C = 32, 112, 112, 256
KTOP = C // 2
NCORES = 8
NPC = B // NCORES          # samples per core
S = H * W_                 # 12544 spatial positions
P = 128                    # partitions
ROWS = S // P              # 98 spatial rows per partition
CH = 14                    # rows per chunk
NCH = ROWS // CH           # 7 chunks
XBUFS = 5                  # f32 x-tile slots (14KB/partition each)
BBUFS = 14                 # bf16 x-tile slots (7KB/partition each)


def build(nc, n_samples=NPC):
    import concourse.tile as tile
    import concourse.mybir as mybir
    from contextlib import ExitStack

    f32 = mybir.dt.float32
    bf16 = mybir.dt.bfloat16
    Alu = mybir.AluOpType

    x_d = nc.dram_tensor("x", [n_samples, H, W_, C], f32, kind="ExternalInput")
    w_d = nc.dram_tensor("W", [C, C], f32, kind="ExternalInput")
    b_d = nc.dram_tensor("b", [C], f32, kind="ExternalInput")
    o_d = nc.dram_tensor("out", [n_samples, H, W_, C], bf16,
                         kind="ExternalOutput")

    # constants baked into the NEFF
    pidx = np.arange(P)[:, None, None] + 128 * np.arange(2)[None, :, None]
    ut_np = (pidx < np.arange(C)[None, None, :]).astype(np.float32)  # [128, 2, 256]
    ut_d = nc.inline_tensor(ut_np, name="ut_const")
    id_d = nc.inline_tensor(np.eye(P, dtype=np.float32), name="id_const")

    x_v = x_d.ap().rearrange("s h w c -> s (h w) c").rearrange(
        "s (p n) c -> s p n c", p=P)
    o_v = o_d.ap().rearrange("s h w c -> s (h w) c").rearrange(
        "s (p n) c -> s p n c", p=P)

    with tile.TileContext(nc) as tc, ExitStack() as ctx:
        cst = ctx.enter_context(tc.tile_pool(name="cst", bufs=1))
        xp = ctx.enter_context(tc.tile_pool(name="xp", bufs=XBUFS))
        xb = ctx.enter_context(tc.tile_pool(name="xb", bufs=BBUFS))
        sm = ctx.enter_context(tc.tile_pool(name="sm", bufs=2))

        ps_pr = ctx.enter_context(tc.tile_pool(name="ps_pr", bufs=1, space="PSUM"))
        ps_t2 = ctx.enter_context(tc.tile_pool(name="ps_t2", bufs=1, space="PSUM"))
        ps_zt0 = ctx.enter_context(tc.tile_pool(name="ps_zt0", bufs=1, space="PSUM"))
        ps_zt1 = ctx.enter_context(tc.tile_pool(name="ps_zt1", bufs=1, space="PSUM"))
        ps_tr = ctx.enter_context(tc.tile_pool(name="ps_tr", bufs=1, space="PSUM"))
        ps_sb = ctx.enter_context(tc.tile_pool(name="ps_sb", bufs=1, space="PSUM"))
        ps_rk = ctx.enter_context(tc.tile_pool(name="ps_rk", bufs=1, space="PSUM"))
        ps_mb = ctx.enter_context(tc.tile_pool(name="ps_mb", bufs=1, space="PSUM"))

        # constants go out on the scalar-engine DMA queue so the first x
        # chunk reads (sync queue) dispatch immediately
        w_sb = cst.tile([P, 2, C], f32)
        nc.scalar.dma_start(w_sb, w_d.ap().rearrange("(h p) c -> p h c", p=P))
        b_sb = cst.tile([P, 2], f32)
        nc.scalar.dma_start(b_sb, b_d.ap().rearrange("(h p) -> p h", p=P))
        ut_sb = cst.tile([P, 2, C], f32)
        nc.scalar.dma_start(ut_sb, ut_d.ap())
        id_sb = cst.tile([P, P], f32)
        nc.scalar.dma_start(id_sb, id_d.ap())
        ones_c = cst.tile([P, 1], f32)
        nc.vector.memset(ones_c, 1.0)
        ones_r = cst.tile([1, P], f32)
        nc.vector.memset(ones_r, 1.0)
        ones_rb = cst.tile([1, P], bf16)
        nc.vector.memset(ones_rb, 1.0)

        pend = None  # deferred (xbs, mb16, s) of the previous sample

        def flush_one(pend, j):
            xbs, mb16, ps = pend
            mb_bc = mb16.unsqueeze(1).broadcast_to([P, CH, C])
            nc.vector.tensor_mul(xbs[j], xbs[j], mb_bc)
            nc.scalar.dma_start(o_v[ps, :, j * CH:(j + 1) * CH, :], xbs[j])

        for s in range(n_samples):
            # ---- A(s): per chunk: load; fold 14 rows -> 7 on DVE; 7 PE
            #      accumulate matmuls; bf16 convert on scalar engine; then
            #      the previous sample's chunk-j mul + store ----
            xbs = []
            pr = ps_pr.tile([1, C], f32, name=f"pr_{s}", tag="pr")
            for j in range(NCH):
                xc = xp.tile([P, CH, C], f32, tag="x", name=f"x_{s}_{j}")
                nc.sync.dma_start(xc, x_v[s, :, j * CH:(j + 1) * CH, :])
                a7 = sm.tile([P, 7, C], f32, name=f"a7_{s}_{j}", tag="a7")
                nc.vector.tensor_add(a7, xc[:, 0:7, :], xc[:, 7:14, :])
                for r in range(7):
                    nc.tensor.matmul(pr, lhsT=ones_c, rhs=a7[:, r, :],
                                     start=(j == 0 and r == 0),
                                     stop=(j == NCH - 1 and r == 6))
                xc16 = xb.tile([P, CH, C], bf16, tag="xb", name=f"xb_{s}_{j}")
                nc.scalar.copy(xc16, xc)
                xbs.append(xc16)
                if pend is not None:
                    flush_one(pend, j)

            # ---- B(s): mask chain ----
            # high_priority: the chain's small ops must not queue behind the
            # NEXT sample's converts on the scalar engine (a convert that
            # waits on its chunk read would head-of-line-block the ready
            # sigmoid/copies, delaying mask(s) and serializing the endgame).
            # NOT for the last sample: there are no next-sample converts to
            # outrun, and hoisting the chain above this sample's final
            # convert would head-of-line-block it instead.
            hp = tc.high_priority(offset=200) if s < n_samples - 1 else None
            if hp is not None:
                hp.__enter__()
            prow = sm.tile([1, C], f32, name=f"prow_{s}", tag="prow")
            nc.scalar.copy(prow, pr)
            t2 = ps_t2.tile([P, 2], f32, name=f"t2_{s}", tag="t2")
            for h in range(2):
                nc.tensor.transpose(t2[:, h:h + 1], prow[:, h * P:(h + 1) * P],
                                    id_sb[0:1, 0:1])
            pts = sm.tile([P, 2], f32, name=f"pts_{s}", tag="pts")
            nc.scalar.copy(pts, t2)

            # gating: zT[co_h] = sum_ci W[ci, co].T @ pooledT
            zt = [ps_zt0.tile([P, 1], f32, name=f"zt0_{s}", tag="zt0"),
                  ps_zt1.tile([P, 1], f32, name=f"zt1_{s}", tag="zt1")]
            for co in range(2):
                for ci in range(2):
                    nc.tensor.matmul(
                        zt[co],
                        lhsT=w_sb[:, ci, co * P:(co + 1) * P],
                        rhs=pts[:, ci:ci + 1],
                        start=(ci == 0),
                        stop=(ci == 1),
                    )
            st = sm.tile([P, 2], f32, name=f"st_{s}", tag="st")
            for h in range(2):
                nc.scalar.activation(
                    st[:, h:h + 1], zt[h],
                    func=mybir.ActivationFunctionType.Sigmoid,
                    bias=b_sb[:, h:h + 1], scale=1.0 / S)

            # scores row form + broadcast across partitions:
            # SB[p, h*128+i] = s[h*128+i]  (per-half transpose so the
            # matmul rhs sits at base partition 0)
            sb_ps = ps_sb.tile([P, C], f32, name=f"sb_{s}", tag="sbb")
            for h in range(2):
                tr_ps = ps_tr.tile([1, P], f32, name=f"trp_{s}_{h}", tag="trp")
                nc.tensor.transpose(tr_ps, st[:, h:h + 1], id_sb)
                tr_sb = sm.tile([1, P], f32, name=f"trs_{s}_{h}", tag="trs")
                nc.scalar.copy(tr_sb, tr_ps)
                nc.tensor.matmul(sb_ps[:, h * P:(h + 1) * P], lhsT=ones_r,
                                 rhs=tr_sb, start=True, stop=True)

            # rank comparisons
            r_sb = sm.tile([P, 2, C], f32, name=f"r_{s}", tag="r")
            eq_sb = sm.tile([P, C], f32, name=f"eq_{s}", tag="eq")
            for h in range(2):
                nc.vector.tensor_scalar(
                    r_sb[:, h, :], sb_ps, st[:, h:h + 1], None, Alu.is_lt)
                nc.vector.tensor_scalar(
                    eq_sb, sb_ps, st[:, h:h + 1], None, Alu.is_equal)
                nc.vector.tensor_mul(eq_sb, eq_sb, ut_sb[:, h, :])
                nc.vector.tensor_add(r_sb[:, h, :], r_sb[:, h, :], eq_sb)

            rk_ps = ps_rk.tile([1, C], f32, name=f"rk_{s}", tag="rk")
            for h in range(2):
                nc.tensor.matmul(rk_ps, lhsT=ones_c, rhs=r_sb[:, h, :],
                                 start=(h == 0), stop=(h == 1))

            # mask row in bf16 ({0,1} exact) so the broadcast matmul streams
            # at 1 cycle/row instead of fp32's 4
            mrow = sm.tile([1, C], bf16, name=f"mrow_{s}", tag="mrow")
            nc.vector.tensor_scalar(mrow, rk_ps, float(KTOP) - 0.5, None, Alu.is_lt)

            mb_ps = ps_mb.tile([P, C], f32, name=f"mb_{s}", tag="mb")
            nc.tensor.matmul(mb_ps, lhsT=ones_rb, rhs=mrow,
                             start=True, stop=True)
            mb16 = sm.tile([P, C], bf16, name=f"mbs_{s}", tag="mbs")
            nc.scalar.copy(mb16, mb_ps)
            if hp is not None:
                hp.__exit__(None, None, None)

            pend = (xbs, mb16, s)

        for j in range(NCH):
            flush_one(pend, j)

    return nc


def make_nc(n_samples=NPC, num_devices=NCORES):
    import concourse.bacc as bacc
    nc = bacc.Bacc("TRN2", target_bir_lowering=False, debug=False,
                   num_devices=num_devices)
    build(nc, n_samples)
    nc.compile()
    return nc


_NC_CACHE = {}


def kernel(x, W, b):
    from concourse import bass_utils
    x = np.ascontiguousarray(x, dtype=np.float32)
    W = np.ascontiguousarray(W, dtype=np.float32)
    b = np.ascontiguousarray(b, dtype=np.float32)
    assert x.shape == (B, H, W_, C)
    if "nc" not in _NC_CACHE:
        _NC_CACHE["nc"] = make_nc()
    nc = _NC_CACHE["nc"]
    in_maps = [
        {"x": x[c * NPC:(c + 1) * NPC], "W": W, "b": b} for c in range(NCORES)
    ]
    # the axon terminal occasionally reports a transient
    # NRT_EXEC_UNIT_UNRECOVERABLE; a retry has always recovered it
    last_err = None
    for _ in range(3):
        try:
            res = bass_utils.run_bass_kernel_spmd(
                nc, in_maps, core_ids=list(range(NCORES)))
            return np.concatenate(
                [r["out"].astype(np.float32) for r in res.results], axis=0)
        except Exception as e:
            last_err = e
    raise last_err
